# revision 8
# baseline (speedup 1.0000x reference)
"""Trainium2 Bass kernel: causal RTF long convolution via FFT (nn_RTF_58660663328946).

Math (validated vs reference in fp64 numpy, ~1e-7 rel):
  y[b,:,d] = first L samples of circular_conv_N(u[b,:,d] zero-padded, ktilde_d),
  where Khat_d(f) = Bhat'_d(w_f)/Ahat_d(w_f),  w_f = 2*pi*f/N, N = 2L = 16384,
  Ahat  = 1 + sum_i a_i e^{-j w i},
  Bhat' = h0 + sum_i (b_i + h0*a_i) e^{-j w i}   (h0 direct-feedthrough folded in).

Device algorithm per core (128 channels):
  Phase K: evaluate Khat(d, f) on all 16384 freqs via K=65 matmuls against a
           host-provided trig Vandermonde E, complex division on DVE/GpSimd/ACT,
           bounce (d,f)->per-channel (k2,k1) grids through DRAM.
  Main:    for each channel d and batch pair (b0,b1): z = u[b0] + i*u[b1];
           two-stage 128x128 decimation FFT done entirely as matmuls
           (data-stationary first stage => no transposes), twiddle and spectral
           multiplies as elementwise ops, inverse likewise; Re/Im of the result
           are y[b0], y[b1].

Sharding: channels d are split 128-per-core across 8 cores. The host reshapes
u to channel-major per shard (pure data layout for DMA efficiency); all FLOPs
run on device.
"""

import numpy as np

NCORES = 8
D = 1024
DC = 128        # channels per core
B = 4
L = 8192
NF = 16384      # FFT length = 2L
NCO = 64        # filter order
FCH = 512       # freq chunk (Phase K)
NCH = NF // FCH

_F32 = None     # set lazily (mybir import)


# ----------------------------------------------------------------------------
# host-side constants
# ----------------------------------------------------------------------------

def _host_consts():
    f64 = np.float64
    t64 = np.arange(64, dtype=f64)
    i128 = np.arange(128, dtype=f64)

    def c(m):
        return np.cos(m).astype(np.float32)

    def s(m):
        return np.sin(m).astype(np.float32)

    # stage A fwd: F64[t1,k1] = e^{-2pi i t1 k1 / 128}, t1 in [0,64)
    angA = 2 * np.pi * np.outer(t64, i128) / 128
    # stage C fwd lhsT: FC[t2,k2] = e^{-2pi i t2 k2 / 128}
    angC = 2 * np.pi * np.outer(i128, i128) / 128
    # fwd twiddle at (t2,k1): W = e^{-2pi i t2 k1 / NF}
    angW = 2 * np.pi * np.outer(i128, i128) / NF

    cn = {}
    cn["Fr64"], cn["Fi64"], cn["Fi64n"] = c(angA), -s(angA), s(angA)
    cn["FCr"], cn["FCi"], cn["FCin"] = c(angC), -s(angC), s(angC)
    # inverse stage A rhs: FB[k2,t2] = e^{+2pi i k2 t2 / 128}
    cn["FBr"], cn["FBi"], cn["FBin"] = c(angC), s(angC), -s(angC)
    # inverse stage C lhsT: FB2[k1,t1] = e^{+2pi i k1 t1 / 128}, t1 in [0,64)
    angB2 = 2 * np.pi * np.outer(i128, t64) / 128
    cn["FB2r"], cn["FB2i"], cn["FB2in"] = c(angB2), s(angB2), -s(angB2)

    def tile4(m):
        return np.tile(m, (1, 4)).copy()

    cn["Wr4"], cn["Wi4"] = tile4(c(angW)), tile4(-s(angW))
    # inverse twiddle at (k1,t2): e^{+2pi i k1 t2 / NF} / NF  (1/N folded here)
    cn["WBr4"] = tile4((np.cos(angW) / NF).astype(np.float32))
    cn["WBi4"] = tile4((np.sin(angW) / NF).astype(np.float32))

    # E (65, NF): rows 0..63 = harmonics i=1..64: e^{-2pi i f i / NF};
    # row 64 = constant term (1, 0)
    iN = np.arange(1, 65, dtype=f64)
    fN = np.arange(NF, dtype=f64)
    angE = 2 * np.pi * np.outer(iN, fN) / NF
    cn["Er"] = np.concatenate([c(angE), np.ones((1, NF), np.float32)])
    cn["Ei"] = np.concatenate([-s(angE), np.zeros((1, NF), np.float32)])

    cn["ident"] = np.eye(128, dtype=np.float32)
    return cn


_CONST_SHAPES = {
    "Fr64": (64, 128), "Fi64": (64, 128), "Fi64n": (64, 128),
    "FCr": (128, 128), "FCi": (128, 128), "FCin": (128, 128),
    "FBr": (128, 128), "FBi": (128, 128), "FBin": (128, 128),
    "FB2r": (128, 64), "FB2i": (128, 64), "FB2in": (128, 64),
    "Wr4": (128, 512), "Wi4": (128, 512),
    "WBr4": (128, 512), "WBi4": (128, 512),
    "ident": (128, 128),
}


# ----------------------------------------------------------------------------
# device program
# ----------------------------------------------------------------------------

def _emit(nc, tc, ctx, io):
    import concourse.bass as bass
    from concourse import mybir
    F32 = mybir.dt.float32
    AF = mybir.ActivationFunctionType
    OP = mybir.AluOpType
    tp = tc.tile_pool

    cp = ctx.enter_context(tp(name="consts", bufs=1))
    cn = {}
    for name, shp in _CONST_SHAPES.items():
        cn[name] = cp.tile(list(shp), F32, tag=name, name=name)
        nc.sync.dma_start(cn[name][:], io[name])

    # ---------------- Phase K: Khat -> kg DRAM ----------------
    with (
        tp(name="kprep", bufs=1) as kc,
        tp(name="epool", bufs=2) as ep,
        tp(name="kpsum", bufs=1, space="PSUM") as kps,
        tp(name="ktmp", bufs=2) as kt,
    ):
        # coefficient matrices C_A, C_B (65, 128):
        # rows 0..63 = coeffs^T (harmonics 1..64), row 64 = 1 / h0 constant term
        ca = kc.tile([65, 128], F32, tag="ca")
        nc.vector.memset(ca[64:65, :], 1.0)
        nc.sync.dma_start(ca[0:64, :], io["aT"])
        cb = kc.tile([65, 128], F32, tag="cb")
        nc.sync.dma_start(cb[64:65, :], io["h0r"])
        a_sb = kc.tile([128, 64], F32, tag="a_sb")
        b_sb = kc.tile([128, 64], F32, tag="b_sb")
        h0c = kc.tile([128, 1], F32, tag="h0c")
        nc.sync.dma_start(a_sb[:], io["a_"])
        nc.sync.dma_start(b_sb[:], io["b_"])
        nc.sync.dma_start(h0c[:], io["h0c"])
        bp = kc.tile([128, 64], F32, tag="bp")
        # bp = a*h0 + b
        nc.vector.scalar_tensor_tensor(
            bp[:], a_sb[:], h0c[:], b_sb[:], op0=OP.mult, op1=OP.add)
        with tp(name="tps", bufs=1, space="PSUM") as tpp:
            ptp = tpp.tile([64, 128], F32, tag="ptp")
            nc.tensor.transpose(ptp[:], bp[:], cn["ident"][:])
            nc.scalar.copy(cb[0:64, :], ptp[:])

        for ch in range(NCH):
            er = ep.tile([65, FCH], F32, tag="er")
            ei = ep.tile([65, FCH], F32, tag="ei")
            nc.sync.dma_start(er[:], io["Er"][:, ch * FCH:(ch + 1) * FCH])
            nc.sync.dma_start(ei[:], io["Ei"][:, ch * FCH:(ch + 1) * FCH])
            Ar = kps.tile([128, FCH], F32, tag="Ar")
            Ai = kps.tile([128, FCH], F32, tag="Ai")
            Br = kps.tile([128, FCH], F32, tag="Br")
            Bi = kps.tile([128, FCH], F32, tag="Bi")
            nc.tensor.matmul(Ar[:], ca[:], er[:], start=True, stop=True)
            nc.tensor.matmul(Ai[:], ca[:], ei[:], start=True, stop=True)
            nc.tensor.matmul(Br[:], cb[:], er[:], start=True, stop=True)
            nc.tensor.matmul(Bi[:], cb[:], ei[:], start=True, stop=True)

            d1 = kt.tile([128, FCH], F32, tag="d1")
            d2 = kt.tile([128, FCH], F32, tag="d2")
            nc.scalar.activation(d1[:], Ar[:], AF.Square)
            nc.scalar.activation(d2[:], Ai[:], AF.Square)
            den = kt.tile([128, FCH], F32, tag="den")
            nc.gpsimd.tensor_add(den[:], d1[:], d2[:])
            rec = kt.tile([128, FCH], F32, tag="rec")
            scr = kt.tile([128, FCH], F32, tag="scr")
            nc.vector.reciprocal_approx_accurate(rec[:], den[:], scr[:])
            brs = kt.tile([128, FCH], F32, tag="brs")
            bis = kt.tile([128, FCH], F32, tag="bis")
            nc.scalar.copy(brs[:], Br[:])
            nc.scalar.copy(bis[:], Bi[:])
            p1 = kt.tile([128, FCH], F32, tag="p1")
            p2 = kt.tile([128, FCH], F32, tag="p2")
            p3 = kt.tile([128, FCH], F32, tag="p3")
            p4 = kt.tile([128, FCH], F32, tag="p4")
            nc.vector.tensor_mul(p1[:], Ar[:], brs[:])
            nc.vector.tensor_mul(p2[:], Ai[:], bis[:])
            nc.vector.tensor_mul(p3[:], Ar[:], bis[:])
            nc.vector.tensor_mul(p4[:], Ai[:], brs[:])
            nr = kt.tile([128, FCH], F32, tag="nr")
            ni = kt.tile([128, FCH], F32, tag="ni")
            nc.gpsimd.tensor_add(nr[:], p1[:], p2[:])
            nc.gpsimd.tensor_sub(ni[:], p3[:], p4[:])
            kr = kt.tile([128, FCH], F32, tag="kr")
            ki = kt.tile([128, FCH], F32, tag="ki")
            nc.vector.tensor_mul(kr[:], nr[:], rec[:])
            nc.gpsimd.tensor_mul(ki[:], ni[:], rec[:])
            kgflat = io["kg"].rearrange("d pl k2 k1 -> d pl (k2 k1)")
            nc.sync.dma_start(kgflat[:, 0, ch * FCH:(ch + 1) * FCH], kr[:])
            nc.sync.dma_start(kgflat[:, 1, ch * FCH:(ch + 1) * FCH], ki[:])

    # ---------------- main loop: 64 sub-groups of (2 channels x 2 pairs) ----
    up = ctx.enter_context(tp(name="upool", bufs=3))
    kgp = ctx.enter_context(tp(name="kgpool", bufs=2))
    ztp = ctx.enter_context(tp(name="ztpool", bufs=2))
    yhp = ctx.enter_context(tp(name="yhpool", bufs=2))
    rpp = ctx.enter_context(tp(name="rppool", bufs=2))
    ysp = ctx.enter_context(tp(name="yspool", bufs=2))
    tw = ctx.enter_context(tp(name="twtmp", bufs=2))
    pYt = ctx.enter_context(tp(name="pYt", bufs=1, space="PSUM"))
    pU = ctx.enter_context(tp(name="pU", bufs=1, space="PSUM"))
    pR = ctx.enter_context(tp(name="pR", bufs=1, space="PSUM"))
    pY = ctx.enter_context(tp(name="pY", bufs=1, space="PSUM"))

    uT = io["uT"]
    yT = io["yT"]
    kg = io["kg"]

    for sg in range(DC // 2):
        d0 = 2 * sg
        ut0 = up.tile([64, 512], F32, tag="ut0")
        ut1 = up.tile([64, 512], F32, tag="ut1")
        nc.sync.dma_start(
            ut0[:].rearrange("p (b t) -> p b t", b=4),
            uT[d0].rearrange("b (p t) -> p b t", p=64))
        nc.sync.dma_start(
            ut1[:].rearrange("p (b t) -> p b t", b=4),
            uT[d0 + 1].rearrange("b (p t) -> p b t", p=64))

        kgr = kgp.tile([128, 512], F32, tag="kgr")
        kgi = kgp.tile([128, 512], F32, tag="kgi")
        for i, dd in enumerate((d0, d0, d0 + 1, d0 + 1)):
            nc.sync.dma_start(kgr[:, i * 128:(i + 1) * 128], kg[dd, 0])
            nc.sync.dma_start(kgi[:, i * 128:(i + 1) * 128], kg[dd, 1])

        # ---- forward stage A (data-stationary: no transposes) ----
        yr = pYt.tile([128, 512], F32, tag="ytr")
        yi = pYt.tile([128, 512], F32, tag="yti")
        for i in range(4):
            ut = ut0 if i < 2 else ut1
            p = i % 2
            Mr = ut[:, (2 * p) * 128:(2 * p + 1) * 128]
            Mi = ut[:, (2 * p + 1) * 128:(2 * p + 2) * 128]
            o = slice(i * 128, (i + 1) * 128)
            nc.tensor.matmul(yr[:, o], Mr, cn["Fr64"][:], start=True, stop=False)
            nc.tensor.matmul(yi[:, o], Mr, cn["Fi64"][:], start=True, stop=False)
            nc.tensor.matmul(yr[:, o], Mi, cn["Fi64n"][:], start=False, stop=True)
            nc.tensor.matmul(yi[:, o], Mi, cn["Fr64"][:], start=False, stop=True)

        # ---- forward twiddle ----
        t1 = tw.tile([128, 512], F32, tag="t1")
        t2 = tw.tile([128, 512], F32, tag="t2")
        t3 = tw.tile([128, 512], F32, tag="t3")
        t4 = tw.tile([128, 512], F32, tag="t4")
        nc.vector.tensor_mul(t1[:], yr[:], cn["Wr4"][:])
        nc.vector.tensor_mul(t2[:], yi[:], cn["Wi4"][:])
        nc.vector.tensor_mul(t3[:], yr[:], cn["Wi4"][:])
        nc.vector.tensor_mul(t4[:], yi[:], cn["Wr4"][:])
        ztr = ztp.tile([128, 512], F32, tag="ztr")
        zti = ztp.tile([128, 512], F32, tag="zti")
        nc.gpsimd.tensor_sub(ztr[:], t1[:], t2[:])
        nc.gpsimd.tensor_add(zti[:], t3[:], t4[:])

        # ---- forward stage C ----
        ur = pU.tile([128, 512], F32, tag="ur")
        ui = pU.tile([128, 512], F32, tag="ui")
        nc.tensor.matmul(ur[:], cn["FCr"][:], ztr[:], start=True, stop=False)
        nc.tensor.matmul(ui[:], cn["FCr"][:], zti[:], start=True, stop=False)
        nc.tensor.matmul(ui[:], cn["FCi"][:], ztr[:], start=False, stop=True)
        nc.tensor.matmul(ur[:], cn["FCin"][:], zti[:], start=False, stop=True)

        # ---- spectral multiply ----
        s1 = tw.tile([128, 512], F32, tag="s1")
        s2 = tw.tile([128, 512], F32, tag="s2")
        s3 = tw.tile([128, 512], F32, tag="s3")
        s4 = tw.tile([128, 512], F32, tag="s4")
        nc.vector.tensor_mul(s1[:], ur[:], kgr[:])
        nc.vector.tensor_mul(s2[:], ui[:], kgi[:])
        nc.vector.tensor_mul(s3[:], ur[:], kgi[:])
        nc.vector.tensor_mul(s4[:], ui[:], kgr[:])
        yhr = yhp.tile([128, 512], F32, tag="yhr")
        yhi = yhp.tile([128, 512], F32, tag="yhi")
        nc.gpsimd.tensor_sub(yhr[:], s1[:], s2[:])
        nc.gpsimd.tensor_add(yhi[:], s3[:], s4[:])

        # ---- inverse stage A (data-stationary) ----
        rr = pR.tile([128, 512], F32, tag="rr")
        ri = pR.tile([128, 512], F32, tag="ri")
        for i in range(4):
            o = slice(i * 128, (i + 1) * 128)
            Yr = yhr[:, o]
            Yi = yhi[:, o]
            nc.tensor.matmul(rr[:, o], Yr, cn["FBr"][:], start=True, stop=False)
            nc.tensor.matmul(ri[:, o], Yr, cn["FBi"][:], start=True, stop=False)
            nc.tensor.matmul(rr[:, o], Yi, cn["FBin"][:], start=False, stop=True)
            nc.tensor.matmul(ri[:, o], Yi, cn["FBr"][:], start=False, stop=True)

        # ---- inverse twiddle (1/N folded into WB consts) ----
        q1 = tw.tile([128, 512], F32, tag="q1")
        q2 = tw.tile([128, 512], F32, tag="q2")
        q3 = tw.tile([128, 512], F32, tag="q3")
        q4 = tw.tile([128, 512], F32, tag="q4")
        nc.vector.tensor_mul(q1[:], rr[:], cn["WBr4"][:])
        nc.vector.tensor_mul(q2[:], ri[:], cn["WBi4"][:])
        nc.vector.tensor_mul(q3[:], rr[:], cn["WBi4"][:])
        nc.vector.tensor_mul(q4[:], ri[:], cn["WBr4"][:])
        rpr = rpp.tile([128, 512], F32, tag="rpr")
        rpi = rpp.tile([128, 512], F32, tag="rpi")
        nc.gpsimd.tensor_sub(rpr[:], q1[:], q2[:])
        nc.gpsimd.tensor_add(rpi[:], q3[:], q4[:])

        # ---- inverse stage C: yr rows 0..63, yi rows 64..127 of one bank ----
        yg = pY.tile([128, 512], F32, tag="yg")
        nc.tensor.matmul(yg[0:64, :], cn["FB2r"][:], rpr[:], start=True, stop=False)
        nc.tensor.matmul(yg[64:128, :], cn["FB2r"][:], rpi[:], start=True, stop=False)
        nc.tensor.matmul(yg[0:64, :], cn["FB2in"][:], rpi[:], start=False, stop=True)
        nc.tensor.matmul(yg[64:128, :], cn["FB2i"][:], rpr[:], start=False, stop=True)

        ysb = ysp.tile([128, 512], F32, tag="ysb")
        nc.scalar.copy(ysb[:], yg[:])
        for i, dd in enumerate((d0, d0, d0 + 1, d0 + 1)):
            p = i % 2
            o = slice(i * 128, (i + 1) * 128)
            nc.sync.dma_start(
                yT[dd, 2 * p].rearrange("(p t) -> p t", p=64), ysb[0:64, o])
            nc.sync.dma_start(
                yT[dd, 2 * p + 1].rearrange("(p t) -> p t", p=64), ysb[64:128, o])


# ----------------------------------------------------------------------------
# program build + host wrapper
# ----------------------------------------------------------------------------

_PROG = None


def _build_program():
    import concourse.bass as bass
    import concourse.tile as tile
    from concourse import bacc, mybir
    from contextlib import ExitStack

    F32 = mybir.dt.float32
    nc = bacc.Bacc("TRN2", target_bir_lowering=False, debug=False)
    io = {}
    io["uT"] = nc.dram_tensor("uT", [DC, B, L], F32, kind="ExternalInput").ap()
    io["aT"] = nc.dram_tensor("aT", [64, 128], F32, kind="ExternalInput").ap()
    io["a_"] = nc.dram_tensor("a_", [128, 64], F32, kind="ExternalInput").ap()
    io["b_"] = nc.dram_tensor("b_", [128, 64], F32, kind="ExternalInput").ap()
    io["h0c"] = nc.dram_tensor("h0c", [128, 1], F32, kind="ExternalInput").ap()
    io["h0r"] = nc.dram_tensor("h0r", [1, 128], F32, kind="ExternalInput").ap()
    io["Er"] = nc.dram_tensor("Er", [65, NF], F32, kind="ExternalInput").ap()
    io["Ei"] = nc.dram_tensor("Ei", [65, NF], F32, kind="ExternalInput").ap()
    for name, shp in _CONST_SHAPES.items():
        io[name] = nc.dram_tensor(name, list(shp), F32, kind="ExternalInput").ap()
    io["kg"] = nc.dram_tensor("kg", [DC, 2, 128, 128], F32).ap()
    io["yT"] = nc.dram_tensor("yT", [DC, B, L], F32, kind="ExternalOutput").ap()

    with tile.TileContext(nc) as tc:
        with ExitStack() as ctx:
            _emit(nc, tc, ctx, io)
    nc.compile()
    return nc


def _get_program():
    global _PROG
    if _PROG is None:
        _PROG = _build_program()
    return _PROG


def make_in_maps(u, ab, h_0):
    u = np.ascontiguousarray(u, dtype=np.float32)
    ab = np.ascontiguousarray(ab, dtype=np.float32)
    h_0 = np.ascontiguousarray(h_0, dtype=np.float32)
    cn = _host_consts()
    in_maps = []
    for c in range(NCORES):
        sl = slice(c * DC, (c + 1) * DC)
        a = ab[sl]                      # (128, 64)
        b = ab[D + c * DC: D + (c + 1) * DC]
        h = h_0[sl]
        m = {
            "uT": np.ascontiguousarray(u[:, :, sl].transpose(2, 0, 1)),
            "aT": np.ascontiguousarray(a.T),
            "a_": a,
            "b_": b,
            "h0c": np.ascontiguousarray(h.reshape(128, 1)),
            "h0r": np.ascontiguousarray(h.reshape(1, 128)),
            "Er": cn["Er"],
            "Ei": cn["Ei"],
        }
        for name in _CONST_SHAPES:
            m[name] = cn[name]
        in_maps.append(m)
    return in_maps


def assemble_output(results, u_dtype=np.float32):
    y = np.empty((B, L, D), dtype=np.float32)
    for c in range(NCORES):
        yT = results[c]["yT"]           # (128, 4, 8192)
        y[:, :, c * DC:(c + 1) * DC] = yT.transpose(1, 2, 0)
    return y.astype(u_dtype, copy=False)


def kernel(u, ab, h_0):
    from concourse.bass_utils import run_bass_kernel_spmd
    nc = _get_program()
    in_maps = make_in_maps(u, ab, h_0)
    res = run_bass_kernel_spmd(nc, in_maps, list(range(NCORES)))
    return assemble_output(res.results, u.dtype)
